# revision 12
# baseline (speedup 1.0000x reference)
"""DescriptorLoss kernel for Trainium2 (8 NeuronCores, SPMD data-parallel).

Math:
    d[b,ij,kl] = sum_c desc0[b,c,ij] * desc1[b,c,kl]
    loss = mean(where(mask, 250*relu(1 - d), relu(d - 0.2)))

Per core (shard = (batch, i-slab) -> 1024 ij rows x 4096 kl cols), the PE
computes d' = 5*d via fp8 DoubleRow matmuls into PSUM fp32 in 32 chunks
of [128 x 1024] (psum pool depth 4).  In d' units the hinges sit at 1
and 5:
    5*loss_elem = relu(d'-1)        if m == 0
                  250*relu(5-d')    if m == 1

22 chunks go to the DVE (one fused custom op per chunk, Src1 = t =
(m ? 8192 : 1) fp8 e5m2):
    body = relu(d' - t) + relu((t - d' - 8187) * 250),  accum = sum
  m=0: relu(d'-1); m=1: 250*relu(5-d').  One PSUM read per element.

10 chunks go to ACT: the PE injects the mask into PSUM
(psum += (-8192*I).T @ m, DoubleRow) giving dM = d' - 8192*m; ACT runs
two relu passes with the 250 weight folded into the free affine:
    acc1 = sum relu(dM - 1)              = sum_{m=0} relu(d'-1)
    acc2 = sum relu(-250*dM - 250*8187)  = 250 * sum_{m=1} relu(5-d')

DoubleRow: contraction K=128 split as [64 partitions, 2 k-planes]; both
operands use k = p + 64*i so the result is exact regardless of the HW
pairing order.  DMA: DVE-chunk masks stream on the sync HWDGE ring in
consumption order (ascending group sizes); descriptors and ACT-inject
masks ride the scalar ring ahead of the ACT hinge work.
"""

import numpy as np
import ml_dtypes
from operator import add

import concourse.bacc as bacc
import concourse.mybir as mybir
import concourse.tile as tile
import concourse.dve_ops as dve_ops_mod
from concourse.dve_spec import Spec, Src0, Src1, C0, C1, relu, lower
from concourse.dve_uop import DveOpSpec
from concourse.bass_utils import run_bass_kernel_spmd

B, D, H, W = 2, 128, 64, 64
N_CORES = 8
IJ = H * W                # 4096
ROWS_PER_CORE = IJ // 4   # 1024
G = ROWS_PER_CORE // 128  # 8 row groups of 128
CH = 1024                 # chunk columns
KT = IJ // CH             # 4 chunks per row group
N_CHUNKS = G * KT         # 32
MOFF = 8192.0             # mask offset (exact in fp8 e5m2)
LAM = 250.0

ACT_CHUNKS = tuple(range(4, N_CHUNKS, 3))          # 10 chunks on ACT
DVE_CHUNKS = tuple(c for c in range(N_CHUNKS) if c not in ACT_CHUNKS)
DVE_GROUPS = (2, 2, 4, 6, 8)                       # sync-ring DMA windows
MACT_SPLIT = 4                                     # ACT chunks in mact0

_cached = {}

_OP_NAME = "HINGE_PAIR_MASKED_ANT"


def _hinge_ref(in0, in1, s0, s1, imm2):
    x = in0.astype(np.float32)
    t = in1.astype(np.float32)
    out = np.maximum(x - t, 0) + np.maximum((t - x - s0) * s1, 0)
    return out, out.reshape(out.shape[0], -1).sum(axis=-1, keepdims=True).astype(
        np.float32
    )


def _register_dve_op():
    """Register the fused two-hinge op in dve_ops.OPS (documented extension
    point; the uop table is emitted per-NEFF at compile time)."""
    for op in dve_ops_mod.OPS:
        if op.name == _OP_NAME:
            return op
    spec = Spec(
        body=relu(Src0 - Src1) + relu((Src1 - Src0 - C0) * C1),
        accum=add,
        reference=_hinge_ref,
    )
    opcode = dve_ops_mod._CUSTOM_DVE_ROW_BASE + len(dve_ops_mod.OPS)
    shas = {}
    for ver in ("v3", "v4"):
        shas[ver] = DveOpSpec(
            name=_OP_NAME, opcode=opcode, uops=lower(spec, ver=ver), rd1_en=True
        ).sha(ver)
    op = dve_ops_mod.DveOp(_OP_NAME, spec, subdim=False, uops_sha=shas)
    dve_ops_mod.OPS.append(op)
    dve_ops_mod._SUB_OPCODE_FOR_NAME[_OP_NAME] = opcode
    dve_ops_mod.CUSTOM_DVE_SPECS[_OP_NAME] = spec
    return op


_HINGE_OP = _register_dve_op()


def _build_program():
    nc = bacc.Bacc("TRN2")
    f32 = mybir.dt.float32
    bf16 = mybir.dt.bfloat16
    f8 = mybir.dt.float8e5
    f8e4 = mybir.dt.float8e4
    Act = mybir.ActivationFunctionType
    DR = mybir.MatmulPerfMode.DoubleRow

    # DoubleRow layouts: k = p + 64*i
    aw = nc.declare_dram_parameter("aw", [64, G * 2, 128], f8e4, isOutput=False)
    bm0 = nc.declare_dram_parameter("bm0", [64, 4, 512], f8e4, isOutput=False)
    bm1 = nc.declare_dram_parameter("bm1", [64, 12, 512], f8e4, isOutput=False)
    idn = nc.declare_dram_parameter("idn", [64, 2, 128], f8, isOutput=False)
    mact0 = nc.declare_dram_parameter(
        "mact0", [64, MACT_SPLIT * 4, 512], f8, isOutput=False)
    mact1 = nc.declare_dram_parameter(
        "mact1", [64, (len(ACT_CHUNKS) - MACT_SPLIT) * 4, 512], f8,
        isOutput=False)
    mvs = [
        nc.declare_dram_parameter(f"mv{i}", [128, n * CH], f8, isOutput=False)
        for i, n in enumerate(DVE_GROUPS)
    ]
    accs_out = nc.declare_dram_parameter("accs", [128, 48], f32, isOutput=True)

    # DVE-chunk -> (group, col offset) map
    dve_loc = {}
    k = 0
    for gq, n in enumerate(DVE_GROUPS):
        for idx in range(n):
            dve_loc[DVE_CHUNKS[k]] = (gq, idx * CH)
            k += 1
    act_idx = {c: j for j, c in enumerate(ACT_CHUNKS)}

    with tile.TileContext(nc) as tc:
        with (
            tc.tile_pool(name="desc", bufs=1) as desc_pool,
            tc.tile_pool(name="mask", bufs=6) as mask_pool,
            tc.tile_pool(name="scr", bufs=4) as scr_pool,
            tc.tile_pool(name="accs", bufs=1) as acc_pool,
            tc.tile_pool(name="psd", bufs=4, space="PSUM") as psum_pool,
        ):
            mgrp = [
                mask_pool.tile([128, n * CH], f8, tag="m", name=f"mg{i}")
                for i, n in enumerate(DVE_GROUPS)
            ]
            a_t = desc_pool.tile([64, G * 2, 128], f8e4, tag="a")
            b0_t = desc_pool.tile([64, 4, 512], f8e4, tag="b0")
            b1_t = desc_pool.tile([64, 12, 512], f8e4, tag="b1")
            id_t = desc_pool.tile([64, 2, 128], f8, tag="idn")
            ma0_t = desc_pool.tile([64, MACT_SPLIT * 4, 512], f8, tag="ma0")
            ma1_t = desc_pool.tile(
                [64, (len(ACT_CHUNKS) - MACT_SPLIT) * 4, 512], f8, tag="ma1")
            warm = desc_pool.tile([128, 8], bf16, tag="warm")
            warm2 = desc_pool.tile([128, 8], bf16, tag="warm2")
            bias_a = desc_pool.tile([128, 1], f32, tag="ba")
            bias_b = desc_pool.tile([128, 1], f32, tag="bb")
            acc_t = acc_pool.tile([128, 48], f32, tag="accs")

            # sync ring: DVE-chunk masks in consumption order, then output
            for i in range(len(DVE_GROUPS)):
                nc.sync.dma_start(mgrp[i][:], mvs[i][:])
            # scalar ring: descriptors + ACT inject masks
            nc.scalar.dma_start(a_t[:], aw[:])
            nc.scalar.dma_start(b0_t[:], bm0[:])
            nc.scalar.dma_start(ma0_t[:], mact0[:])
            nc.scalar.dma_start(b1_t[:], bm1[:])
            nc.scalar.dma_start(id_t[:], idn[:])
            nc.scalar.dma_start(ma1_t[:], mact1[:])

            nc.vector.memset(warm[:], 0.0)
            nc.vector.memset(bias_a[:], -1.0)
            nc.vector.memset(bias_b[:], -(LAM * (MOFF - 5.0)))
            nc.vector.memset(acc_t[:, 42:], 0.0)
            # prime the ACT relu table (~2.7us one-time) under the input DMAs
            nc.scalar.activation(warm2[:], warm[:], Act.Relu, bias=bias_a[:], scale=1.0)

            n_dve = 0
            for cid in range(N_CHUNKS):
                on_act = cid in ACT_CHUNKS
                g, h = divmod(cid, KT)
                psum_d = psum_pool.tile([128, CH], f32, tag="d")
                for s in range(2):
                    blk = 2 * h + s
                    bt = b0_t if blk < 2 else b1_t
                    boff = blk * 2 if blk < 2 else (blk - 2) * 2
                    nc.tensor.matmul(
                        psum_d[:, s * 512:(s + 1) * 512],
                        a_t[:, g * 2:(g + 1) * 2, :],
                        bt[:, boff:boff + 2, :],
                        start=True, stop=not on_act, perf_mode=DR,
                    )
                if on_act:
                    j = act_idx[cid]
                    mt = ma0_t if j < MACT_SPLIT else ma1_t
                    moff = j * 4 if j < MACT_SPLIT else (j - MACT_SPLIT) * 4
                    for s in range(2):
                        nc.tensor.matmul(
                            psum_d[:, s * 512:(s + 1) * 512],
                            id_t[:],
                            mt[:, moff + 2 * s:moff + 2 * s + 2, :],
                            start=False, stop=True, perf_mode=DR,
                        )
                    scr1 = scr_pool.tile([128, CH], bf16, tag="scr")
                    scr2 = scr_pool.tile([128, CH], bf16, tag="scr")
                    c0 = 22 + 2 * j
                    nc.scalar.activation(
                        scr1[:], psum_d[:], Act.Relu,
                        bias=bias_a[:], scale=1.0,
                        accum_out=acc_t[:, c0:c0 + 1],
                    )
                    nc.scalar.activation(
                        scr2[:], psum_d[:], Act.Relu,
                        bias=bias_b[:], scale=-LAM,
                        accum_out=acc_t[:, c0 + 1:c0 + 2],
                    )
                else:
                    gq, mcol = dve_loc[cid]
                    scr = scr_pool.tile([128, CH], bf16, tag="scr")
                    nc.vector._custom_dve(
                        _HINGE_OP,
                        out=scr[:], in0=psum_d[:], in1=mgrp[gq][:, mcol:mcol + CH],
                        s0=MOFF - 5.0, s1=LAM,
                        accum_out=acc_t[:, n_dve:n_dve + 1],
                    )
                    n_dve += 1

            nc.sync.dma_start(accs_out[:], acc_t[:])

    nc.finalize()
    return nc


def _to_dr(x128):
    """[128, C] -> [64, C/512 blocks, 2, 512] DoubleRow layout, k = p + 64*i."""
    c = x128.shape[1]
    return np.ascontiguousarray(
        x128.reshape(2, 64, c // 512, 512).transpose(1, 2, 0, 3)
    )


def _prep_inputs(descriptors_0, descriptors_1, similarity_mask):
    d0 = np.asarray(descriptors_0, dtype=np.float32)
    d1 = np.asarray(descriptors_1, dtype=np.float32)
    mkv = np.asarray(similarity_mask)
    idn_128 = (-MOFF * np.eye(D, dtype=np.float32)).astype(ml_dtypes.float8_e5m2)
    idn_dr = np.ascontiguousarray(idn_128.reshape(2, 64, 128).transpose(1, 0, 2))
    in_maps = []
    n_act = len(ACT_CHUNKS)
    for c in range(N_CORES):
        b = c >> 2
        isl = (c & 3) * 16
        aw128 = (
            d0[b].reshape(D, IJ)[:, isl * W:(isl + 16) * W] * np.float32(5.0)
        ).astype(ml_dtypes.float8_e4m3)
        # aw: [64, G, 2, 128]
        aw_dr = np.ascontiguousarray(
            aw128.reshape(2, 64, G, 128).transpose(1, 2, 0, 3)
        )
        bm128 = d1[b].reshape(D, IJ).astype(ml_dtypes.float8_e4m3)
        bm_dr = _to_dr(bm128)  # [64, 8, 2, 512]
        m = mkv[b, isl:isl + 16].reshape(ROWS_PER_CORE, IJ)
        mq = m.reshape(G, 128, KT, CH).transpose(0, 2, 1, 3).reshape(N_CHUNKS, 128, CH)
        # DVE masks: t-form {1, 8192} fp8e5m2, grouped in consumption order
        mvv = {}
        ds = [np.where(mq[cid], np.float32(MOFF), np.float32(1.0)).astype(
            ml_dtypes.float8_e5m2) for cid in DVE_CHUNKS]
        off = 0
        for i, n in enumerate(DVE_GROUPS):
            grp = np.stack(ds[off:off + n])  # [n, 128, CH]
            mvv[f"mv{i}"] = np.ascontiguousarray(
                grp.transpose(1, 0, 2).reshape(128, n * CH)
            )
            off += n
        # ACT masks: {0,1} fp8e5m2 in DoubleRow layout [64, nact, blk, i, 512]
        mact = np.stack([
            _to_dr(mq[cid].astype(ml_dtypes.float8_e5m2)) for cid in ACT_CHUNKS
        ])  # [n_act, 64, 2, 2, 512]
        mact = np.ascontiguousarray(mact.transpose(1, 0, 2, 3, 4))
        im = {
            "aw": np.ascontiguousarray(aw_dr.reshape(64, G * 2, 128)),
            "bm0": np.ascontiguousarray(bm_dr[:, :2].reshape(64, 4, 512)),
            "bm1": np.ascontiguousarray(bm_dr[:, 2:].reshape(64, 12, 512)),
            "idn": np.ascontiguousarray(idn_dr),
            "mact0": np.ascontiguousarray(
                mact[:, :MACT_SPLIT].reshape(64, MACT_SPLIT * 4, 512)),
            "mact1": np.ascontiguousarray(
                mact[:, MACT_SPLIT:].reshape(64, (n_act - MACT_SPLIT) * 4, 512)),
        }
        im.update(mvv)
        in_maps.append(im)
    return in_maps


def _run(in_maps, **kwargs):
    if "nc" not in _cached:
        _cached["nc"] = _build_program()
    return run_bass_kernel_spmd(_cached["nc"], in_maps, list(range(N_CORES)), **kwargs)


def _combine(results):
    total = 0.0
    for r in results:
        accs = r["accs"].astype(np.float64)
        total += accs[:, :42].sum()
    return np.float32(total / 5.0 / float(B * IJ * IJ))


def kernel(descriptors_0, descriptors_1, similarity_mask):
    in_maps = _prep_inputs(descriptors_0, descriptors_1, similarity_mask)
    res = _run(in_maps)
    return _combine(res.results)


# revision 13
# speedup vs baseline: 1.4283x; 1.4283x over previous
"""DescriptorLoss kernel for Trainium2 (8 NeuronCores, SPMD data-parallel).

Math:
    d[b,ij,kl] = sum_c desc0[b,c,ij] * desc1[b,c,kl]
    loss = mean(where(mask, 250*relu(1 - d), relu(d - 0.2)))

Per core (shard = (batch, i-slab) -> 1024 ij rows x 4096 kl cols), the PE
computes d' = 5*d via fp8 matmuls into PSUM fp32 in 32 chunks of
[128 x 1024] (psum pool depth 4).  In d' units the hinges sit at 1 and 5:
    5*loss_elem = relu(d'-1)        if m == 0
                  250*relu(5-d')    if m == 1

23 chunks go to the DVE (one fused custom op per chunk, Src1 = t =
(m ? 8192 : 1) fp8 e5m2):
    body = relu(d' - t) + relu((t - d' - 8187) * 250),  accum = sum
  m=0: relu(d'-1); m=1: 250*relu(5-d').  One PSUM read per element.

9 chunks go to ACT: the PE injects the mask into PSUM
(psum += (-8192*I).T @ m) giving dM = d' - 8192*m; ACT runs two relu
passes with the 250 weight folded into the free affine:
    acc1 = sum relu(dM - 1)              = sum_{m=0} relu(d'-1)
    acc2 = sum relu(-250*dM - 250*8187)  = 250 * sum_{m=1} relu(5-d')

Scheduling notes (learned from traces):
  - Exactly 8 input DMAs: the Tile scheduler has 8 DMA-completion
    semaphore lanes; more input DMAs alias lanes and create false
    multi-microsecond waits on unrelated transfers.
  - DVE-chunk masks stream on the sync HWDGE ring in consumption order
    with ascending group sizes; descriptors lead, ACT-inject masks ride
    the scalar ring ahead of the ACT hinge work.
  - No PE warmup: cold matmul rate (~1us/chunk) still outpaces the DVE
    hinge stream (~1.4us/chunk), and sustained real matmuls open the
    HAM clock gate on their own.
"""

import numpy as np
import ml_dtypes
from operator import add

import concourse.bacc as bacc
import concourse.mybir as mybir
import concourse.tile as tile
import concourse.dve_ops as dve_ops_mod
from concourse.dve_spec import Spec, Src0, Src1, C0, C1, relu, lower
from concourse.dve_uop import DveOpSpec
from concourse.bass_utils import run_bass_kernel_spmd

B, D, H, W = 2, 128, 64, 64
N_CORES = 8
IJ = H * W                # 4096
ROWS_PER_CORE = IJ // 4   # 1024
G = ROWS_PER_CORE // 128  # 8 row groups of 128
CH = 1024                 # chunk columns
KT = IJ // CH             # 4 chunks per row group
N_CHUNKS = G * KT         # 32
MOFF = 8192.0             # mask offset (exact in fp8 e5m2)
LAM = 250.0

ACT_CHUNKS = (4, 7, 11, 14, 18, 21, 25, 28, 31)    # 9 chunks on ACT
DVE_CHUNKS = tuple(c for c in range(N_CHUNKS) if c not in ACT_CHUNKS)
DVE_GROUPS = (4, 4, 7, 8)                          # sync-ring DMA windows
MACT_SPLIT = 2                                     # ACT chunks in first window

_cached = {}

_OP_NAME = "HINGE_PAIR_MASKED_ANT"


def _hinge_ref(in0, in1, s0, s1, imm2):
    x = in0.astype(np.float32)
    t = in1.astype(np.float32)
    out = np.maximum(x - t, 0) + np.maximum((t - x - s0) * s1, 0)
    return out, out.reshape(out.shape[0], -1).sum(axis=-1, keepdims=True).astype(
        np.float32
    )


def _register_dve_op():
    """Register the fused two-hinge op in dve_ops.OPS (documented extension
    point; the uop table is emitted per-NEFF at compile time)."""
    for op in dve_ops_mod.OPS:
        if op.name == _OP_NAME:
            return op
    spec = Spec(
        body=relu(Src0 - Src1) + relu((Src1 - Src0 - C0) * C1),
        accum=add,
        reference=_hinge_ref,
    )
    opcode = dve_ops_mod._CUSTOM_DVE_ROW_BASE + len(dve_ops_mod.OPS)
    shas = {}
    for ver in ("v3", "v4"):
        shas[ver] = DveOpSpec(
            name=_OP_NAME, opcode=opcode, uops=lower(spec, ver=ver), rd1_en=True
        ).sha(ver)
    op = dve_ops_mod.DveOp(_OP_NAME, spec, subdim=False, uops_sha=shas)
    dve_ops_mod.OPS.append(op)
    dve_ops_mod._SUB_OPCODE_FOR_NAME[_OP_NAME] = opcode
    dve_ops_mod.CUSTOM_DVE_SPECS[_OP_NAME] = spec
    return op


_HINGE_OP = _register_dve_op()


def _build_program():
    nc = bacc.Bacc("TRN2")
    f32 = mybir.dt.float32
    bf16 = mybir.dt.bfloat16
    f8 = mybir.dt.float8e5
    f8e4 = mybir.dt.float8e4
    Act = mybir.ActivationFunctionType
    n_act = len(ACT_CHUNKS)

    # ab0 = aw [128,1024] ++ b columns 0-1023; b1 = b columns 1024-4095
    ab0 = nc.declare_dram_parameter("ab0", [D, 2 * CH], f8e4, isOutput=False)
    b1 = nc.declare_dram_parameter("b1", [D, 3 * CH], f8e4, isOutput=False)
    # mi0 = idn [128,128] ++ first MACT_SPLIT ACT masks; mi1 = the rest
    mi0 = nc.declare_dram_parameter(
        "mi0", [D, D + MACT_SPLIT * CH], f8, isOutput=False)
    mi1 = nc.declare_dram_parameter(
        "mi1", [D, (n_act - MACT_SPLIT) * CH], f8, isOutput=False)
    mvs = [
        nc.declare_dram_parameter(f"mv{i}", [128, n * CH], f8, isOutput=False)
        for i, n in enumerate(DVE_GROUPS)
    ]
    accs_out = nc.declare_dram_parameter("accs", [128, 64], f32, isOutput=True)

    # DVE-chunk -> (group, col offset); ACT-chunk -> index
    dve_loc = {}
    k = 0
    for gq, n in enumerate(DVE_GROUPS):
        for idx in range(n):
            dve_loc[DVE_CHUNKS[k]] = (gq, idx * CH)
            k += 1
    act_idx = {c: j for j, c in enumerate(ACT_CHUNKS)}

    with tile.TileContext(nc) as tc:
        with (
            tc.tile_pool(name="desc", bufs=1) as desc_pool,
            tc.tile_pool(name="mask", bufs=6) as mask_pool,
            tc.tile_pool(name="scr", bufs=4) as scr_pool,
            tc.tile_pool(name="accs", bufs=1) as acc_pool,
            tc.tile_pool(name="psd", bufs=4, space="PSUM") as psum_pool,
        ):
            mgrp = [
                mask_pool.tile([128, n * CH], f8, tag="m", name=f"mg{i}")
                for i, n in enumerate(DVE_GROUPS)
            ]
            ab0_t = desc_pool.tile([D, 2 * CH], f8e4, tag="ab0")
            b1_t = desc_pool.tile([D, 3 * CH], f8e4, tag="b1")
            mi0_t = desc_pool.tile([D, D + MACT_SPLIT * CH], f8, tag="mi0")
            mi1_t = desc_pool.tile([D, (n_act - MACT_SPLIT) * CH], f8, tag="mi1")
            warm = desc_pool.tile([128, 8], bf16, tag="warm")
            warm2 = desc_pool.tile([128, 8], bf16, tag="warm2")
            bias_a = desc_pool.tile([128, 1], f32, tag="ba")
            bias_b = desc_pool.tile([128, 1], f32, tag="bb")
            accD_t = acc_pool.tile([128, 32], f32, tag="accD")
            accA_t = acc_pool.tile([128, 32], f32, tag="accA")

            # sync ring: descriptors, then DVE masks in consumption order
            nc.sync.dma_start(ab0_t[:], ab0[:])
            nc.sync.dma_start(b1_t[:], b1[:])
            for i in range(len(DVE_GROUPS)):
                nc.sync.dma_start(mgrp[i][:], mvs[i][:])
            # scalar ring: idn + ACT inject masks
            nc.scalar.dma_start(mi0_t[:], mi0[:])
            nc.scalar.dma_start(mi1_t[:], mi1[:])

            nc.vector.memset(warm[:], 0.0)
            nc.vector.memset(bias_a[:], -1.0)
            nc.vector.memset(bias_b[:], -(LAM * (MOFF - 5.0)))
            nc.vector.memset(accD_t[:, 23:], 0.0)
            nc.vector.memset(accA_t[:, 2 * n_act:], 0.0)
            # prime the ACT relu table (~2.7us one-time) under the input DMAs
            nc.scalar.activation(warm2[:], warm[:], Act.Relu, bias=bias_a[:], scale=1.0)

            n_dve = 0
            for cid in range(N_CHUNKS):
                on_act = cid in ACT_CHUNKS
                g, h = divmod(cid, KT)
                psum_d = psum_pool.tile([128, CH], f32, tag="d")
                for s in range(2):
                    col = h * CH + s * 512
                    rhs = (ab0_t[:, CH + col:CH + col + 512] if col < CH
                           else b1_t[:, col - CH:col - CH + 512])
                    nc.tensor.matmul(
                        psum_d[:, s * 512:(s + 1) * 512],
                        ab0_t[:, g * 128:(g + 1) * 128], rhs,
                        start=True, stop=not on_act,
                    )
                if on_act:
                    j = act_idx[cid]
                    for s in range(2):
                        if j < MACT_SPLIT:
                            mrhs = mi0_t[:, D + j * CH + s * 512:D + j * CH + s * 512 + 512]
                        else:
                            jj = j - MACT_SPLIT
                            mrhs = mi1_t[:, jj * CH + s * 512:jj * CH + s * 512 + 512]
                        nc.tensor.matmul(
                            psum_d[:, s * 512:(s + 1) * 512],
                            mi0_t[:, 0:D], mrhs,
                            start=False, stop=True,
                        )
                    scr1 = scr_pool.tile([128, CH], bf16, tag="scr")
                    scr2 = scr_pool.tile([128, CH], bf16, tag="scr")
                    nc.scalar.activation(
                        scr1[:], psum_d[:], Act.Relu,
                        bias=bias_a[:], scale=1.0,
                        accum_out=accA_t[:, 2 * j:2 * j + 1],
                    )
                    nc.scalar.activation(
                        scr2[:], psum_d[:], Act.Relu,
                        bias=bias_b[:], scale=-LAM,
                        accum_out=accA_t[:, 2 * j + 1:2 * j + 2],
                    )
                else:
                    gq, mcol = dve_loc[cid]
                    scr = scr_pool.tile([128, CH], bf16, tag="scr")
                    nc.vector._custom_dve(
                        _HINGE_OP,
                        out=scr[:], in0=psum_d[:], in1=mgrp[gq][:, mcol:mcol + CH],
                        s0=MOFF - 5.0, s1=LAM,
                        accum_out=accD_t[:, n_dve:n_dve + 1],
                    )
                    n_dve += 1

            nc.sync.dma_start(accs_out[:, :32], accD_t[:])
            nc.sync.dma_start(accs_out[:, 32:], accA_t[:])

    nc.finalize()
    return nc


def _prep_inputs(descriptors_0, descriptors_1, similarity_mask):
    d0 = np.asarray(descriptors_0, dtype=np.float32)
    d1 = np.asarray(descriptors_1, dtype=np.float32)
    mkv = np.asarray(similarity_mask)
    idn128 = (-MOFF * np.eye(D, dtype=np.float32)).astype(ml_dtypes.float8_e5m2)
    in_maps = []
    n_act = len(ACT_CHUNKS)
    for c in range(N_CORES):
        b = c >> 2
        isl = (c & 3) * 16
        aw128 = (
            d0[b].reshape(D, IJ)[:, isl * W:(isl + 16) * W] * np.float32(5.0)
        ).astype(ml_dtypes.float8_e4m3)
        bm128 = d1[b].reshape(D, IJ).astype(ml_dtypes.float8_e4m3)
        m = mkv[b, isl:isl + 16].reshape(ROWS_PER_CORE, IJ)
        mq = m.reshape(G, 128, KT, CH).transpose(0, 2, 1, 3).reshape(N_CHUNKS, 128, CH)
        # DVE masks: t-form {1, 8192} fp8e5m2, grouped in consumption order
        mvv = {}
        ds = [np.where(mq[cid], np.float32(MOFF), np.float32(1.0)).astype(
            ml_dtypes.float8_e5m2) for cid in DVE_CHUNKS]
        off = 0
        for i, n in enumerate(DVE_GROUPS):
            grp = np.stack(ds[off:off + n])  # [n, 128, CH]
            mvv[f"mv{i}"] = np.ascontiguousarray(
                grp.transpose(1, 0, 2).reshape(128, n * CH)
            )
            off += n
        # ACT masks: {0,1} fp8e5m2
        mact = np.stack([mq[cid].astype(ml_dtypes.float8_e5m2)
                         for cid in ACT_CHUNKS])  # [n_act, 128, CH]
        mact = mact.transpose(1, 0, 2)            # [128, n_act, CH]
        im = {
            "ab0": np.ascontiguousarray(
                np.concatenate([aw128, bm128[:, :CH]], axis=1)),
            "b1": np.ascontiguousarray(bm128[:, CH:]),
            "mi0": np.ascontiguousarray(np.concatenate(
                [idn128, mact[:, :MACT_SPLIT].reshape(128, MACT_SPLIT * CH)],
                axis=1)),
            "mi1": np.ascontiguousarray(
                mact[:, MACT_SPLIT:].reshape(128, (n_act - MACT_SPLIT) * CH)),
        }
        im.update(mvv)
        in_maps.append(im)
    return in_maps


def _run(in_maps, **kwargs):
    if "nc" not in _cached:
        _cached["nc"] = _build_program()
    return run_bass_kernel_spmd(_cached["nc"], in_maps, list(range(N_CORES)), **kwargs)


def _combine(results):
    total = 0.0
    n_act = len(ACT_CHUNKS)
    for r in results:
        accs = r["accs"].astype(np.float64)
        total += accs[:, :23].sum() + accs[:, 32:32 + 2 * n_act].sum()
    return np.float32(total / 5.0 / float(B * IJ * IJ))


def kernel(descriptors_0, descriptors_1, similarity_mask):
    in_maps = _prep_inputs(descriptors_0, descriptors_1, similarity_mask)
    res = _run(in_maps)
    return _combine(res.results)


# revision 14
# speedup vs baseline: 1.6124x; 1.1289x over previous
"""DescriptorLoss kernel for Trainium2 (8 NeuronCores, SPMD data-parallel).

Math:
    d[b,ij,kl] = sum_c desc0[b,c,ij] * desc1[b,c,kl]
    loss = mean(where(mask, 250*relu(1 - d), relu(d - 0.2)))

Per core (shard = (batch, i-slab) -> 1024 ij rows x 4096 kl cols), the PE
computes d' = 5*d via fp8 matmuls into PSUM fp32 in 32 chunks of
[128 x 1024] (psum pool depth 4).  In d' units the hinges sit at 1 and 5:
    5*loss_elem = relu(d'-1)        if m == 0
                  250*relu(5-d')    if m == 1

23 chunks go to the DVE (one fused custom op per chunk, Src1 = t =
(m ? 8192 : 1) fp8 e5m2):
    body = relu(d' - t) + relu((t - d' - 8187) * 250),  accum = sum
  m=0: relu(d'-1); m=1: 250*relu(5-d').  One PSUM read per element.

9 chunks go to ACT: the PE injects the mask into PSUM
(psum += (-8192*I).T @ m) giving dM = d' - 8192*m; ACT runs two relu
passes with the 250 weight folded into the free affine:
    acc1 = sum relu(dM - 1)              = sum_{m=0} relu(d'-1)
    acc2 = sum relu(-250*dM - 250*8187)  = 250 * sum_{m=1} relu(5-d')

Scheduling notes (learned from traces):
  - Chunks are processed h-minor (all h=0 column blocks first), so the
    single leading DMA (aw ++ b[:, :1024] ++ first two DVE masks) feeds
    the first 8 matmuls and the hinge pipeline starts ~1us after data
    lands.  All transfers ride ONE sync-HWDGE ring in exact consumption
    order: packet-level round-robin between rings/queues would otherwise
    delay early transfers by later ones.
  - Exactly 8 input DMAs: the Tile scheduler has 8 DMA-completion
    semaphore lanes; more input DMAs alias lanes and create false
    multi-microsecond waits on unrelated transfers.
  - The scalar engine issues no DMAs (descriptor generation costs
    ~0.7us/DMA on the issuing engine) so ACT is free for hinge passes.
  - No PE warmup: cold matmul rate still outpaces the DVE hinge stream,
    and sustained real matmuls open the HAM clock gate on their own.
"""

import numpy as np
import ml_dtypes
from operator import add

import concourse.bacc as bacc
import concourse.mybir as mybir
import concourse.tile as tile
import concourse.dve_ops as dve_ops_mod
from concourse.dve_spec import Spec, Src0, Src1, C0, C1, relu, lower
from concourse.dve_uop import DveOpSpec
from concourse.bass_utils import run_bass_kernel_spmd

B, D, H, W = 2, 128, 64, 64
N_CORES = 8
IJ = H * W                # 4096
ROWS_PER_CORE = IJ // 4   # 1024
G = ROWS_PER_CORE // 128  # 8 row groups of 128
CH = 1024                 # chunk columns
KT = IJ // CH             # 4 chunks per row group
N_CHUNKS = G * KT         # 32
MOFF = 8192.0             # mask offset (exact in fp8 e5m2)
LAM = 250.0

# processing order: h-minor (all h=0 chunks first), g-major within h
ORDER = tuple(g * KT + h for h in range(KT) for g in range(G))
ACT_POS = (3, 6, 10, 13, 17, 20, 24, 27, 30)     # positions on ACT (9)
DVE_POS = tuple(p for p in range(N_CHUNKS) if p not in ACT_POS)
# DVE mask windows, in processing order: 2 chunks ride in the lead DMA
DVE_GROUPS = (2, 3, 6, 6, 6)
MACT_SPLIT = 2                                   # ACT chunks in mi0

_cached = {}

_OP_NAME = "HINGE_PAIR_MASKED_ANT"


def _hinge_ref(in0, in1, s0, s1, imm2):
    x = in0.astype(np.float32)
    t = in1.astype(np.float32)
    out = np.maximum(x - t, 0) + np.maximum((t - x - s0) * s1, 0)
    return out, out.reshape(out.shape[0], -1).sum(axis=-1, keepdims=True).astype(
        np.float32
    )


def _register_dve_op():
    """Register the fused two-hinge op in dve_ops.OPS (documented extension
    point; the uop table is emitted per-NEFF at compile time)."""
    for op in dve_ops_mod.OPS:
        if op.name == _OP_NAME:
            return op
    spec = Spec(
        body=relu(Src0 - Src1) + relu((Src1 - Src0 - C0) * C1),
        accum=add,
        reference=_hinge_ref,
    )
    opcode = dve_ops_mod._CUSTOM_DVE_ROW_BASE + len(dve_ops_mod.OPS)
    shas = {}
    for ver in ("v3", "v4"):
        shas[ver] = DveOpSpec(
            name=_OP_NAME, opcode=opcode, uops=lower(spec, ver=ver), rd1_en=True
        ).sha(ver)
    op = dve_ops_mod.DveOp(_OP_NAME, spec, subdim=False, uops_sha=shas)
    dve_ops_mod.OPS.append(op)
    dve_ops_mod._SUB_OPCODE_FOR_NAME[_OP_NAME] = opcode
    dve_ops_mod.CUSTOM_DVE_SPECS[_OP_NAME] = spec
    return op


_HINGE_OP = _register_dve_op()


def _build_program():
    nc = bacc.Bacc("TRN2")
    f32 = mybir.dt.float32
    bf16 = mybir.dt.bfloat16
    f8 = mybir.dt.float8e5
    f8e4 = mybir.dt.float8e4
    Act = mybir.ActivationFunctionType
    n_act = len(ACT_POS)

    # lead = aw [128,1024] ++ b[:, :1024] ++ masks for the first 2 DVE chunks
    lead = nc.declare_dram_parameter("lead", [D, 4 * CH], f8, isOutput=False)
    b1 = nc.declare_dram_parameter("b1", [D, 3 * CH], f8e4, isOutput=False)
    mi0 = nc.declare_dram_parameter(
        "mi0", [D, D + MACT_SPLIT * CH], f8, isOutput=False)
    mi1 = nc.declare_dram_parameter(
        "mi1", [D, (n_act - MACT_SPLIT) * CH], f8, isOutput=False)
    mvs = [
        nc.declare_dram_parameter(f"mv{i}", [128, n * CH], f8, isOutput=False)
        for i, n in enumerate(DVE_GROUPS[1:])
    ]
    accs_out = nc.declare_dram_parameter("accs", [128, 64], f32, isOutput=True)

    # DVE position -> (window, col offset); window -1 = lead DMA
    dve_loc = {}
    k = 0
    for gq, n in enumerate(DVE_GROUPS):
        for idx in range(n):
            dve_loc[DVE_POS[k]] = (gq - 1, idx * CH)
            k += 1
    act_idx = {p: j for j, p in enumerate(ACT_POS)}

    with tile.TileContext(nc) as tc:
        with (
            tc.tile_pool(name="desc", bufs=1) as desc_pool,
            tc.tile_pool(name="mask", bufs=6) as mask_pool,
            tc.tile_pool(name="scr", bufs=4) as scr_pool,
            tc.tile_pool(name="accs", bufs=1) as acc_pool,
            tc.tile_pool(name="psd", bufs=4, space="PSUM") as psum_pool,
        ):
            lead_t = desc_pool.tile([D, 4 * CH], f8, tag="lead")
            b1_t = desc_pool.tile([D, 3 * CH], f8e4, tag="b1")
            mi0_t = desc_pool.tile([D, D + MACT_SPLIT * CH], f8, tag="mi0")
            mi1_t = desc_pool.tile([D, (n_act - MACT_SPLIT) * CH], f8, tag="mi1")
            mgrp = [
                mask_pool.tile([128, n * CH], f8, tag="m", name=f"mg{i}")
                for i, n in enumerate(DVE_GROUPS[1:])
            ]
            warm = desc_pool.tile([128, 8], bf16, tag="warm")
            warm2 = desc_pool.tile([128, 8], bf16, tag="warm2")
            bias_a = desc_pool.tile([128, 1], f32, tag="ba")
            bias_b = desc_pool.tile([128, 1], f32, tag="bb")
            accD_t = acc_pool.tile([128, 32], f32, tag="accD")
            accA_t = acc_pool.tile([128, 32], f32, tag="accA")

            # single sync ring, exact consumption order
            nc.sync.dma_start(lead_t[:], lead[:])
            nc.sync.dma_start(mgrp[0][:], mvs[0][:])
            nc.sync.dma_start(mi0_t[:], mi0[:])
            nc.sync.dma_start(b1_t[:], b1[:])
            nc.sync.dma_start(mgrp[1][:], mvs[1][:])
            nc.sync.dma_start(mi1_t[:], mi1[:])
            nc.sync.dma_start(mgrp[2][:], mvs[2][:])
            nc.sync.dma_start(mgrp[3][:], mvs[3][:])

            nc.vector.memset(warm[:], 0.0)
            nc.vector.memset(bias_a[:], -1.0)
            nc.vector.memset(bias_b[:], -(LAM * (MOFF - 5.0)))
            nc.vector.memset(accD_t[:, 23:], 0.0)
            nc.vector.memset(accA_t[:, 2 * n_act:], 0.0)
            # prime the ACT relu table (~2.7us one-time) under the input DMAs
            nc.scalar.activation(warm2[:], warm[:], Act.Relu, bias=bias_a[:], scale=1.0)

            aw_all = lead_t[:, 0:CH].bitcast(f8e4)
            b0 = lead_t[:, CH:2 * CH].bitcast(f8e4)

            n_dve = 0
            for pos in range(N_CHUNKS):
                cid = ORDER[pos]
                on_act = pos in ACT_POS
                g, h = divmod(cid, KT)
                psum_d = psum_pool.tile([128, CH], f32, tag="d")
                for s in range(2):
                    if h == 0:
                        rhs = b0[:, s * 512:(s + 1) * 512]
                    else:
                        c0 = (h - 1) * CH + s * 512
                        rhs = b1_t[:, c0:c0 + 512]
                    nc.tensor.matmul(
                        psum_d[:, s * 512:(s + 1) * 512],
                        aw_all[:, g * 128:(g + 1) * 128], rhs,
                        start=True, stop=not on_act,
                    )
                if on_act:
                    j = act_idx[pos]
                    for s in range(2):
                        if j < MACT_SPLIT:
                            mrhs = mi0_t[:, D + j * CH + s * 512:D + j * CH + s * 512 + 512]
                        else:
                            jj = j - MACT_SPLIT
                            mrhs = mi1_t[:, jj * CH + s * 512:jj * CH + s * 512 + 512]
                        nc.tensor.matmul(
                            psum_d[:, s * 512:(s + 1) * 512],
                            mi0_t[:, 0:D], mrhs,
                            start=False, stop=True,
                        )
                    scr1 = scr_pool.tile([128, CH], bf16, tag="scr")
                    scr2 = scr_pool.tile([128, CH], bf16, tag="scr")
                    nc.scalar.activation(
                        scr1[:], psum_d[:], Act.Relu,
                        bias=bias_a[:], scale=1.0,
                        accum_out=accA_t[:, 2 * j:2 * j + 1],
                    )
                    nc.scalar.activation(
                        scr2[:], psum_d[:], Act.Relu,
                        bias=bias_b[:], scale=-LAM,
                        accum_out=accA_t[:, 2 * j + 1:2 * j + 2],
                    )
                else:
                    gq, mcol = dve_loc[pos]
                    src1 = (lead_t[:, 2 * CH + mcol:2 * CH + mcol + CH] if gq < 0
                            else mgrp[gq][:, mcol:mcol + CH])
                    scr = scr_pool.tile([128, CH], bf16, tag="scr")
                    nc.vector._custom_dve(
                        _HINGE_OP,
                        out=scr[:], in0=psum_d[:], in1=src1,
                        s0=MOFF - 5.0, s1=LAM,
                        accum_out=accD_t[:, n_dve:n_dve + 1],
                    )
                    n_dve += 1

            nc.sync.dma_start(accs_out[:, :32], accD_t[:])
            nc.sync.dma_start(accs_out[:, 32:], accA_t[:])

    nc.finalize()
    return nc


def _prep_inputs(descriptors_0, descriptors_1, similarity_mask):
    d0 = np.asarray(descriptors_0, dtype=np.float32)
    d1 = np.asarray(descriptors_1, dtype=np.float32)
    mkv = np.asarray(similarity_mask)
    idn128 = (-MOFF * np.eye(D, dtype=np.float32)).astype(ml_dtypes.float8_e5m2)
    in_maps = []
    n_act = len(ACT_POS)
    for c in range(N_CORES):
        b = c >> 2
        isl = (c & 3) * 16
        aw128 = (
            d0[b].reshape(D, IJ)[:, isl * W:(isl + 16) * W] * np.float32(5.0)
        ).astype(ml_dtypes.float8_e4m3)
        bm128 = d1[b].reshape(D, IJ).astype(ml_dtypes.float8_e4m3)
        m = mkv[b, isl:isl + 16].reshape(ROWS_PER_CORE, IJ)
        mq = m.reshape(G, 128, KT, CH).transpose(0, 2, 1, 3).reshape(N_CHUNKS, 128, CH)
        # DVE masks: t-form {1, 8192} fp8e5m2, in processing order
        dm = [np.where(mq[ORDER[p]], np.float32(MOFF), np.float32(1.0)).astype(
            ml_dtypes.float8_e5m2) for p in DVE_POS]
        mvv = {}
        off = DVE_GROUPS[0]
        for i, n in enumerate(DVE_GROUPS[1:]):
            grp = np.stack(dm[off:off + n])  # [n, 128, CH]
            mvv[f"mv{i}"] = np.ascontiguousarray(
                grp.transpose(1, 0, 2).reshape(128, n * CH)
            )
            off += n
        # ACT masks: {0,1} fp8e5m2, in processing order
        mact = np.stack([mq[ORDER[p]].astype(ml_dtypes.float8_e5m2)
                         for p in ACT_POS])   # [n_act, 128, CH]
        mact = mact.transpose(1, 0, 2)        # [128, n_act, CH]
        lead_np = np.concatenate(
            [aw128.view(ml_dtypes.float8_e5m2),
             bm128[:, :CH].view(ml_dtypes.float8_e5m2),
             dm[0], dm[1]], axis=1)
        im = {
            "lead": np.ascontiguousarray(lead_np),
            "b1": np.ascontiguousarray(bm128[:, CH:]),
            "mi0": np.ascontiguousarray(np.concatenate(
                [idn128, mact[:, :MACT_SPLIT].reshape(128, MACT_SPLIT * CH)],
                axis=1)),
            "mi1": np.ascontiguousarray(
                mact[:, MACT_SPLIT:].reshape(128, (n_act - MACT_SPLIT) * CH)),
        }
        im.update(mvv)
        in_maps.append(im)
    return in_maps


def _run(in_maps, **kwargs):
    if "nc" not in _cached:
        _cached["nc"] = _build_program()
    return run_bass_kernel_spmd(_cached["nc"], in_maps, list(range(N_CORES)), **kwargs)


def _combine(results):
    total = 0.0
    n_act = len(ACT_POS)
    for r in results:
        accs = r["accs"].astype(np.float64)
        total += accs[:, :23].sum() + accs[:, 32:32 + 2 * n_act].sum()
    return np.float32(total / 5.0 / float(B * IJ * IJ))


def kernel(descriptors_0, descriptors_1, similarity_mask):
    in_maps = _prep_inputs(descriptors_0, descriptors_1, similarity_mask)
    res = _run(in_maps)
    return _combine(res.results)
